# revision 65
# baseline (speedup 1.0000x reference)
"""Dilated MHSA block (B=2, N=2048, D=1024, H=16, band +/-16 step 2) on 8 NeuronCores.

Sharding: row-parallel. Each core owns 512 consecutive query rows of one batch
(2 batches x 4 row-blocks = 8 cores) plus a 16-token halo of keys/values on
each side. No collectives; outputs concatenate.

Per-core pipeline (all matmuls in float32r -- full-rate fp32 on the PE):
  A) QKV projection. q,k produced feature-major ([d, tokens], "T layout"),
     v token-major with a ones-column appended per head (for the softmax
     denominator).
  B) QK norm: sum-of-squares over the 64 head dims via a ones-block matmul
     (partition reduction), rsqrt-ish on ACT/DVE, broadcast back over
     partitions via a selector matmul, normalize in place.
  C) Attention per (query-chunk of 256, head): scoresT = k_hat^T q_hat with
     keys on partitions, exp on ACT (no max subtraction needed: |score|<=1),
     0/1 band-mask multiply, attn@V as a matmul contracting keys on
     partitions (no transposes anywhere). Ones-column yields the softmax
     denominator; reciprocal broadcast through a rank-1 matmul; normalize.
  D) Output projection from the head-pair-major normalized attention output.
"""

import sys

sys.path.insert(0, "/opt/trn_rl_repo")

from contextlib import ExitStack

import numpy as np

import concourse.bass as bass
import concourse.tile as tile
from concourse import bacc, mybir
from concourse.bass_utils import run_bass_kernel_spmd

F32 = mybir.dt.float32
F32R = mybir.dt.float32r
BF16 = mybir.dt.bfloat16
AF = mybir.ActivationFunctionType

NB, NSEQ, DMODEL = 2, 2048, 1024
NH, DH = 16, 64
HALO = 16          # k * dilation
DILATION = 2
NCORES = 8
ROWS = NSEQ * NB // NCORES   # 512 query rows per core
TLOC = ROWS + 2 * HALO       # 544 local tokens
NTT = 5                      # token tiles: 4x128 + 32
QC = 256                     # query chunk
NQC = ROWS // QC             # 2
EPS2 = 1e-12                 # guards 1/sqrt(0) on zero-padded halo tokens

# When True, phase A (x, w_qkv) and the output projection (out_hat, w_out) run
# in bf16 (halves weight DMA; ~2e-3 rel err). When False, everything is fp32r
# (~3e-4 rel err). Attention proper is fp32r either way.
WEIGHTS_BF16 = False
WDT = BF16 if WEIGHTS_BF16 else F32R

# Bisect flags
import os
USE_LNEXP = os.environ.get("K_LNEXP", "1") == "1"   # rsqrt via exp(-0.5 ln) + single ACT table
GPSIMD_DMA = os.environ.get("K_GPDMA", "1") == "1"  # constants via Pool SWDGE queue


def _emit(ctx, tc, xT, wqkvT, woutT, bqk, bv, bout, mask, sel2_dram, out):
    nc = tc.nc
    consts = ctx.enter_context(tc.tile_pool(name="consts", bufs=1))
    big = ctx.enter_context(tc.tile_pool(name="big", bufs=1))

    # --- constants -----------------------------------------------------
    U32 = mybir.dt.uint32
    ONE = 0x3F800000  # 1.0f bit pattern (valid fp32r: low 12 bits zero)
    onespair = consts.tile([128, 2], F32R)   # per-head partition-sum weights
    nc.vector.memset(onespair.bitcast(U32), 0)
    nc.vector.memset(onespair[0:64, 0:1].bitcast(U32), ONE)
    nc.vector.memset(onespair[64:128, 1:2].bitcast(U32), ONE)
    sel2 = consts.tile([2, 128], F32R)       # partition broadcast selector
    (nc.gpsimd if GPSIMD_DMA else nc.sync).dma_start(sel2, sel2_dram)
    ones64 = consts.tile([1, 64], F32R)
    nc.vector.memset(ones64.bitcast(U32), ONE)
    eps12 = consts.tile([2, 1], F32)
    nc.vector.memset(eps12, EPS2)
    bqk_sb = consts.tile([128, 16], F32)    # [partition within f-tile, f-tile]
    (nc.gpsimd if GPSIMD_DMA else nc.sync).dma_start(
        bqk_sb, bass.AP(tensor=bqk.tensor, offset=bqk.offset, ap=[[1, 128], [128, 16]])
    )

    # --- resident tensors (DMA order = SP execution order: critical first)
    xT_holder = {}  # filled inside the phase-A scope (freed afterwards)
    mask_sb = big.tile([128, NQC, 3, QC], BF16)     # 0/1 band mask
    (nc.gpsimd if GPSIMD_DMA else nc.sync).dma_start(mask_sb, mask.rearrange("a b p q -> p a b q"))
    bv_sb = consts.tile([128, DMODEL], F32)
    (nc.gpsimd if GPSIMD_DMA else nc.sync).dma_start(
        bv_sb, bass.AP(tensor=bv.tensor, offset=bv.offset, ap=[[0, 128], [1, DMODEL]])
    )
    bout_sb = consts.tile([128, DMODEL], F32)
    wout_sb = big.tile([128, 8, DMODEL], WDT)       # w_out^T, D-major

    qk_sb = big.tile([128, 16, TLOC], F32R)         # q_hat^T / k_hat^T (f-tiles)
    v_aug = big.tile([128, NTT, NH, DH + 1], F32R)  # token-major v + ones col
    nc.vector.memset(v_aug[:, :, :, DH : DH + 1].bitcast(U32), ONE)
    out_hat = big.tile([128, 8, NQC, QC], WDT)      # normalized attn out^T

    # attention-phase SBUF pools are allocated up front so attention emission
    # can overlap phase A's tail (a later pool would wait for space release)
    et_pool = ctx.enter_context(tc.tile_pool(name="et", bufs=2))
    rd_pool = ctx.enter_context(tc.tile_pool(name="rd", bufs=4))
    avs_pool = ctx.enter_context(tc.tile_pool(name="avs", bufs=2))
    osb_pool = ctx.enter_context(tc.tile_pool(name="osb", bufs=2))
    P = {"et": et_pool, "rd": rd_pool, "avs": avs_pool, "osb": osb_pool}

    def qk_pair(hp):
        """QKV projection + QK-norm for q f-tile hp and k f-tile 8+hp.

        q is computed only for the 512 real query tokens (chunks of 256 at
        local offset HALO); k needs the full 544-token halo (chunks of 272).
        """
        wt = P["wqk"].tile([128, 8, 2, 128], WDT, tag="wqk")
        for side in range(2):
            f0 = (hp + 8 * side) * 128
            nc.sync.dma_start(
                wt[:, :, side, :],
                wqkvT[:, f0 : f0 + 128].rearrange("(k p) f -> p k f", p=128),
            )
        for side in range(2):
            ft = hp + 8 * side
            cw = 256 if side == 0 else 272   # q: 2x256 queries, k: 2x272 tokens
            off = HALO if side == 0 else 0
            for ch in range(2):
                sl = slice(off + ch * cw, off + (ch + 1) * cw)
                ps_full = P["qkps"].tile([128, 272], F32, tag="qkps", name=f"ps{hp}")
                ps = ps_full[:, 0:cw]
                for k in range(8):
                    nc.tensor.matmul(
                        ps, wt[:, k, side, :], xT_holder["t"][:, k, sl],
                        start=(k == 0), stop=(k == 7),
                    )
                raw = qk_sb[:, ft, sl]
                nc.vector.tensor_scalar_add(raw, ps, bqk_sb[:, ft : ft + 1])
                sq_full = P["sq"].tile([128, 272], F32R, tag="sq")
                sq = sq_full[:, 0:cw]
                nc.scalar.activation(sq, ps, AF.Square, bias=bqk_sb[:, ft : ft + 1])
                ss_full = P["ssps"].tile([2, 272], F32, tag="ssps", name=f"ss{hp}")
                ss = ss_full[:, 0:cw]
                nc.tensor.matmul(ss, onespair, sq, start=True, stop=True)
                # 1/sqrt(ss+eps) = exp(-0.5*ln(ss+eps)): ln/exp/identity/square
                # share one ACT table set (see _restrict_act_tables), so the
                # kernel runs without mid-stream table reloads.
                sn_full = P["sn"].tile([2, 272], F32, tag="sn")
                sn = sn_full[:, 0:cw]
                inv_full = P["sn"].tile([2, 272], F32R, tag="inv")
                inv = inv_full[:, 0:cw]
                if USE_LNEXP:
                    nc.scalar.activation(sn, ss, AF.Ln, bias=eps12)
                    nc.scalar.activation(inv, sn, AF.Exp, scale=-0.5)
                else:
                    nc.scalar.activation(sn, ss, AF.Sqrt, bias=eps12)
                    with nc.allow_low_precision(reason="fp32r feeds PE"):
                        nc.vector.reciprocal(inv, sn)
                bc_full = P["bcps"].tile([128, 272], F32, tag="bcps", name=f"bc{hp}")
                bc = bc_full[:, 0:cw]
                nc.tensor.matmul(bc, sel2, inv, start=True, stop=True)
                nc.vector.tensor_mul(raw, raw, bc)  # in-place normalize

    def v_chunk(c):
        """V projection for feature chunk c (heads 8c..8c+8) into v_aug."""
        wv = P["wv"].tile([128, 8, 512], WDT, tag="wv")
        nc.sync.dma_start(
            wv,
            wqkvT[:, 2048 + c * 512 : 2048 + (c + 1) * 512].rearrange(
                "(k p) f -> p k f", p=128
            ),
        )
        for tt in range(NTT):
            pt = 128 if tt < 4 else TLOC - 512
            vp = P["vps"].tile([128, 512], F32, tag="vps")
            for k in range(8):
                nc.tensor.matmul(
                    vp[0:pt, :],
                    xT_holder["t"][:, k, tt * 128 : tt * 128 + pt],
                    wv[:, k, :],
                    start=(k == 0), stop=(k == 7),
                )
            nc.vector.tensor_add(
                v_aug[0:pt, tt, c * 8 : (c + 1) * 8, 0:DH],
                vp[0:pt, :].rearrange("p (h d) -> p h d", d=DH),
                bv_sb[0:pt, c * 512 : (c + 1) * 512].rearrange("p (h d) -> p h d", d=DH),
            )

    def attn(h, qc):
        """Banded attention for head h, query chunk qc."""
        ftq, ftk, pb = h // 2, 8 + h // 2, 64 * (h % 2)
        q_ap = qk_sb[pb : pb + 64, ftq, HALO + qc * QC : HALO + qc * QC + QC]
        sc = P["scps"].tile([128, 3, QC], F32, tag="sc")
        for j in range(3):
            kw = 128 if j < 2 else 32
            kj0 = qc * QC + j * 128
            nc.tensor.matmul(
                sc[0:kw, j, :],
                qk_sb[pb : pb + 64, ftk, kj0 : kj0 + kw],
                q_ap,
                start=True, stop=True,
            )
        et = P["et"].tile([128, 3, QC], F32R, tag="et")
        nc.scalar.activation(et[:, 0:2, :], sc[:, 0:2, :], AF.Exp)
        nc.scalar.activation(et[0:32, 2, :], sc[0:32, 2, :], AF.Exp)
        # mask multiply on the otherwise-idle GPSIMD (SBUF-only engine)
        nc.gpsimd.tensor_mul(et[:, 0:2, :], et[:, 0:2, :], mask_sb[:, qc, 0:2, :])
        nc.gpsimd.tensor_mul(et[0:32, 2, :], et[0:32, 2, :], mask_sb[0:32, qc, 2, :])
        av = P["avps"].tile([DH + 1, QC], F32, tag="av")
        for j in range(3):
            kw = 128 if j < 2 else 32
            nc.tensor.matmul(
                av,
                v_aug[0:kw, qc * 2 + j, h, :],
                et[0:kw, j, :],
                start=(j == 0), stop=(j == 2),
            )
        rd = P["rd"].tile([1, QC], F32R, tag="rd")
        with nc.allow_low_precision(reason="fp32r feeds PE"):
            nc.vector.reciprocal(rd, av[DH : DH + 1, :])
        bcr = P["bcrps"].tile([64, QC], F32, tag="bcr")
        nc.tensor.matmul(bcr, ones64, rd, start=True, stop=True)
        avs = P["avs"].tile([64, QC], F32, tag="avs")
        nc.vector.tensor_copy(avs, av[0:DH, :])
        nc.vector.tensor_mul(out_hat[pb : pb + 64, h // 2, qc, :], avs, bcr)

    def proj(qc, ts_):
        """Output projection for one 128-row tile of query chunk qc."""
        for ec in range(2):
            po = P["pops"].tile([128, 512], F32, tag="po")
            for ph in range(8):
                nc.tensor.matmul(
                    po,
                    out_hat[:, ph, qc, ts_ * 128 : (ts_ + 1) * 128],
                    wout_sb[:, ph, ec * 512 : (ec + 1) * 512],
                    start=(ph == 0), stop=(ph == 7),
                )
            osb = P["osb"].tile([128, 512], F32, tag="osb")
            nc.vector.tensor_add(osb, po, bout_sb[:, ec * 512 : (ec + 1) * 512])
            row0 = qc * QC + ts_ * 128
            nc.sync.dma_start(out[row0 : row0 + 128, ec * 512 : (ec + 1) * 512], osb)

    # ---- Phase A: QKV + norm (scoped SBUF + PSUM pools) ---------------
    with (
        tc.tile_pool(name="xtp", bufs=1) as _xt,
        tc.tile_pool(name="wqk", bufs=2) as _wqk,
        tc.tile_pool(name="wv", bufs=2) as _wv,
        tc.tile_pool(name="sq", bufs=2) as _sq,
        tc.tile_pool(name="sn", bufs=4) as _sn,
        tc.tile_pool(name="qkps", bufs=3, space="PSUM") as _qk,
        tc.tile_pool(name="ssps", bufs=1, space="PSUM") as _ss,
        tc.tile_pool(name="bcps", bufs=2, space="PSUM") as _bc,
        tc.tile_pool(name="vps", bufs=2, space="PSUM") as _vp,
    ):
        P.update(wqk=_wqk, wv=_wv, sq=_sq, sn=_sn,
                 qkps=_qk, ssps=_ss, bcps=_bc, vps=_vp)
        xT_sb = _xt.tile([128, 8, TLOC], WDT)       # x^T, d_in-major
        xT_holder["t"] = xT_sb
        xT_r = xT.rearrange("(k p) t -> p k t", p=128)
        nc.sync.dma_start(xT_sb[:, 0:4, :], xT_r[:, 0:4, :])
        nc.sync.dma_start(xT_sb[:, 4:8, :], xT_r[:, 4:8, :])
        # PE warm-up: dummy matmul chain on memset constants during the
        # input-DMA lead-in, so the HAM clock gate (and the cost model's
        # p-state ramp) reaches full rate before the first real matmul.
        warm_rhs = P["sq"].tile([128, 272], F32R, tag="sq", name="warm")
        nc.vector.memset(warm_rhs.bitcast(mybir.dt.uint32), ONE)
        warm_ps = P["ssps"].tile([2, 272], F32, tag="ssps", name="warmps")
        for _ in range(14):
            nc.tensor.matmul(warm_ps, onespair, warm_rhs, start=True, stop=True)
        for hp in range(8):
            qk_pair(hp)
        v_chunk(0)
        v_chunk(1)

    # w_out / b_out loads: SP runs them during the attention phase
    nc.sync.dma_start(wout_sb, woutT.rearrange("(k p) e -> p k e", p=128))
    nc.sync.dma_start(
        bout_sb,
        bass.AP(tensor=bout.tensor, offset=bout.offset, ap=[[0, 128], [1, DMODEL]]),
    )

    # ---- Phases C+D: attention + projection, interleaved per chunk ----
    with (
        tc.tile_pool(name="scps", bufs=2, space="PSUM") as _sc,
        tc.tile_pool(name="avps", bufs=1, space="PSUM") as _av,
        tc.tile_pool(name="bcrps", bufs=1, space="PSUM") as _bcr,
        tc.tile_pool(name="pops", bufs=2, space="PSUM") as _po,
    ):
        P.update(scps=_sc, avps=_av, bcrps=_bcr, pops=_po)
        for qc in range(NQC):
            for h in range(NH):
                attn(h, qc)
            for ts_ in range(2):
                proj(qc, ts_)


def _restrict_act_tables():
    """Restrict the ACT table registry to natural_log_exp_and_others, which
    holds every activation this kernel uses (ln/exp/identity/square/copy).
    The default chooser pairs Ln and Exp with different sets, forcing a
    ~1.3us table reload on every ln<->exp alternation."""
    import concourse.hw_specs as hw_specs
    import concourse.bass_interp as bass_interp

    if getattr(_restrict_act_tables, "done", False):
        return
    orig = hw_specs.get_activation_tables

    def only_lnexp(arch):
        # Keep the full set list (set ids index act_info.json, which walrus
        # also reads), but make natural_log_exp_and_others the only set that
        # offers Ln or Exp so the load-placement pass picks it for both.
        t = orig(arch)
        ln = mybir.ActivationFunctionType.Ln
        ex = mybir.ActivationFunctionType.Exp
        out = {}
        for name, funcs in t.items():
            if name != "natural_log_exp_and_others":
                funcs = funcs - {ln, ex}
            out[name] = funcs
        return out

    hw_specs.get_activation_tables = only_lnexp
    bacc.get_activation_tables = only_lnexp
    bass_interp.get_activation_tables = only_lnexp
    _restrict_act_tables.done = True


def build_nc():
    if USE_LNEXP:
        _restrict_act_tables()
    nc = bacc.Bacc(
        "TRN2", target_bir_lowering=False, debug=False, num_devices=NCORES
    )
    xT = nc.dram_tensor("xT", [DMODEL, TLOC], WDT, kind="ExternalInput").ap()
    wqkvT = nc.dram_tensor("wqkvT", [DMODEL, 3 * DMODEL], WDT, kind="ExternalInput").ap()
    woutT = nc.dram_tensor("woutT", [DMODEL, DMODEL], WDT, kind="ExternalInput").ap()
    bqk = nc.dram_tensor("bqk", [2 * DMODEL], F32, kind="ExternalInput").ap()
    bv = nc.dram_tensor("bv", [DMODEL], F32, kind="ExternalInput").ap()
    bout = nc.dram_tensor("bout", [DMODEL], F32, kind="ExternalInput").ap()
    mask = nc.dram_tensor("mask", [NQC, 3, 128, QC], BF16, kind="ExternalInput").ap()
    sel2 = nc.dram_tensor("sel2", [2, 128], F32R, kind="ExternalInput").ap()
    out = nc.dram_tensor("out", [ROWS, DMODEL], F32, kind="ExternalOutput").ap()
    with tile.TileContext(nc) as tc, ExitStack() as ctx:
        _emit(ctx, tc, xT, wqkvT, woutT, bqk, bv, bout, mask, sel2, out)
    nc.compile()
    return nc


_CACHE = {}


def _get_nc():
    if "nc" not in _CACHE:
        _CACHE["nc"] = build_nc()
    return _CACHE["nc"]


def _round_fp32r(a):
    """Round fp32 array to the fp32r encoding (11-bit mantissa, low 12 bits 0),
    round-to-nearest-even, matching the PE's operand rounding."""
    u = np.ascontiguousarray(a, np.float32).view(np.uint32).copy()
    rem = u & np.uint32(0xFFF)
    base = u & np.uint32(0xFFFFF000)
    lsb = (u >> np.uint32(12)) & np.uint32(1)
    round_up = (rem > 0x800) | ((rem == 0x800) & (lsb == 1))
    out = base + (round_up.astype(np.uint32) << np.uint32(12))
    return out.view(np.float32)


def _core_mask(n0):
    m = np.zeros((NQC, 3, 128, QC), np.float32)
    p = np.arange(128)[:, None]
    qi = np.arange(QC)[None, :]
    for qc in range(NQC):
        for j in range(3):
            jg = n0 - HALO + qc * QC + j * 128 + p  # global key index
            ig = n0 + qc * QC + qi                  # global query index
            d = ig - jg
            ok = (np.abs(d) <= HALO) & (d % DILATION == 0) & (jg >= 0) & (jg < NSEQ)
            m[qc, j] = ok
    return m


def _prep_in_maps(x, w_qkv, b_qkv, w_out, b_out):
    import ml_dtypes

    def wcast(a):
        a = np.ascontiguousarray(a)
        return a.astype(ml_dtypes.bfloat16) if WEIGHTS_BF16 else _round_fp32r(a)

    x = np.asarray(x, np.float32)
    wqkvT = wcast(np.asarray(w_qkv, np.float32).T)
    woutT = wcast(np.asarray(w_out, np.float32).T)
    bqk = np.ascontiguousarray(np.asarray(b_qkv, np.float32)[: 2 * DMODEL])
    bv = np.ascontiguousarray(np.asarray(b_qkv, np.float32)[2 * DMODEL :])
    bout = np.ascontiguousarray(np.asarray(b_out, np.float32))
    sel2 = np.zeros((2, 128), np.float32)
    sel2[0, 0:64] = 1.0
    sel2[1, 64:128] = 1.0
    in_maps = []
    for c in range(NCORES):
        b, n0 = c // 4, (c % 4) * ROWS
        lo, hi = n0 - HALO, n0 + ROWS + HALO
        xs = np.zeros((TLOC, DMODEL), np.float32)
        src_lo, src_hi = max(lo, 0), min(hi, NSEQ)
        xs[src_lo - lo : src_hi - lo] = x[b, src_lo:src_hi]
        in_maps.append(
            {
                "xT": wcast(xs.T),
                "wqkvT": wqkvT,
                "woutT": woutT,
                "bqk": bqk,
                "bv": bv,
                "bout": bout,
                "mask": _core_mask(n0).astype(ml_dtypes.bfloat16),
                "sel2": sel2,
            }
        )
    return in_maps


def run(inputs, trace=False):
    """Returns (full_output, BassKernelResults)."""
    nc = _get_nc()
    in_maps = _prep_in_maps(**inputs)
    res = run_bass_kernel_spmd(nc, in_maps, list(range(NCORES)), trace=trace)
    out = np.empty((NB, NSEQ, DMODEL), np.float32)
    for c in range(NCORES):
        b, n0 = c // 4, (c % 4) * ROWS
        out[b, n0 : n0 + ROWS] = res.results[c]["out"]
    return out, res


def kernel(x, w_qkv, b_qkv, w_out, b_out):
    out, _ = run(
        dict(x=x, w_qkv=w_qkv, b_qkv=b_qkv, w_out=w_out, b_out=b_out), trace=False
    )
    return out
